# revision 12
# baseline (speedup 1.0000x reference)
"""Trainium2 Bass kernel for the non-local (self-attention over spatial
positions) block.

Per batch b (8 batches -> one per NeuronCore):
    xf    = x[b]                       [C=128, N=4096]
    theta = w_theta @ xf               [64, N]
    phi   = w_phi   @ xf               [64, N]
    g     = w_g     @ xf               [64, N]
    attn  = softmax(theta^T phi)       [N, N]   (softmax over keys m)
    y     = g @ attn^T                 [64, N]
    out   = w_last @ y + xf            [128, N]

Design (per core):
 - scoresT orientation: scoresT[m, q] = sum_k phi[k,m] theta[k,q] with phi
   m-tiles stationary; exp(scoresT) feeds the y matmul directly as the
   moving operand (no transposes).
 - exp is the single-engine bottleneck (N*N = 16.7M elems/core) so it is
   SPLIT across two engines per 16-tile chunk: 9 tiles on ACT (table exp,
   bf16 out) and 7 on DVE via a Schraudolph bit-trick: bf16 bits of
   ~exp(x) are round(128*log2e*x + B16) computed by one tensor_scalar
   (f32 PSUM -> int16 SBUF) and bitcast to bf16. The approximation's
   constant scale factor cancels in softmax; rel err ~3% on those tiles,
   end-to-end ~1e-2 < 2e-2 gate.
 - No max-subtraction: logits within +-75; exp fits f32/bf16 range and
   the bit-trick constants are valid to |x|<88.
 - Row sums via a ones column appended to gT (stationary of the y matmul);
   reciprocal via the fast custom-DVE approx on [1,512]; broadcast and
   residual-add offloaded to GPSIMD.
 - Projections produce theta/phi pre-duplicated into both row halves via
   duplicated-column weights, so score matmuls for two m-tiles run
   concurrently in disjoint PE row groups.
"""

import sys

import numpy as np

for _p in ("/opt/trn_rl_repo",):
    if _p not in sys.path:
        sys.path.insert(0, _p)

import concourse.bass as bass
from concourse import bacc
import concourse.mybir as mybir
import concourse.tile as tile
from concourse.bass_utils import run_bass_kernel_spmd

F32 = mybir.dt.float32
F32R = mybir.dt.float32r
BF16 = mybir.dt.bfloat16
I16 = mybir.dt.int16

P = 128     # channels C / partition dim
CB = 64     # bottleneck channels
NQ = 4096   # spatial positions (64*64)
QT = 1024   # probs tile width used for PSUM score tiles
MT = 32     # m (key) tiles of 128

LOG2E = 1.4426950408889634
S16 = 128.0 * LOG2E
B16 = 127.0 * 128.0 - 5.60   # Schraudolph bias tuned for min max-rel-err

# per-chunk tile assignment: which of the 16 score tiles go to the DVE
# (alternating 7/6 per chunk to balance ACT vs DVE+epilogue load)
DVE_SET = frozenset((1, 3, 5, 7, 9, 11, 13))
DVE_SET2 = frozenset((1, 3, 5, 7, 9, 11))

_NC_CACHE = {}


def _build():
    nc = bacc.Bacc()
    x_in = nc.declare_dram_parameter("xb", [P, NQ], F32, isOutput=False)
    wqa_in = nc.declare_dram_parameter("wqa", [P, P], F32, isOutput=False)
    wqb_in = nc.declare_dram_parameter("wqb", [P, P], F32, isOutput=False)
    wg_in = nc.declare_dram_parameter("wgT", [P, CB], F32, isOutput=False)
    wl_in = nc.declare_dram_parameter("wl", [CB, P], F32, isOutput=False)
    out_d = nc.declare_dram_parameter("out", [P, NQ], F32, isOutput=True)

    with tile.TileContext(nc) as tc:
        with (
            tc.tile_pool(name="const", bufs=1) as const,
            tc.tile_pool(name="big", bufs=1) as big,
            tc.tile_pool(name="work", bufs=2) as work,
            tc.tile_pool(name="probs", bufs=8) as probs,
            tc.tile_pool(name="spool", bufs=3, space="PSUM") as spool,
            tc.tile_pool(name="ypool", bufs=1, space="PSUM") as ypool,
            tc.tile_pool(name="opool", bufs=1, space="PSUM") as opool,
        ):
            # ---- loads ----
            xb = big.tile([P, NQ], F32)
            for j in range(8):
                nc.sync.dma_start(
                    out=xb[:, j * 512:(j + 1) * 512],
                    in_=x_in[:, j * 512:(j + 1) * 512],
                )
            wqa = const.tile([P, P], F32)
            wqb = const.tile([P, P], F32)
            wg = const.tile([P, CB], F32)
            wl = const.tile([CB, P], F32)
            nc.sync.dma_start(out=wqa, in_=wqa_in[:, :])
            nc.sync.dma_start(out=wqb, in_=wqb_in[:, :])
            nc.sync.dma_start(out=wg, in_=wg_in[:, :])
            nc.sync.dma_start(out=wl, in_=wl_in[:, :])
            wlr = const.tile([CB, P], F32R)
            nc.vector.tensor_copy(wlr, wl)

            theta = big.tile([P, NQ], F32R)
            phi = big.tile([P, NQ], F32R)

            # ---- projections: wqa/wqb = [w^T | w^T] duplicate theta/phi
            # into both row halves so score matmuls for two m-tiles can run
            # concurrently in disjoint PE row groups. phi copies on ACT,
            # theta copies on DVE so the prologue drains two engines. ----
            for j in range(8):
                js = slice(j * 512, (j + 1) * 512)
                ps = spool.tile([P, QT], F32, tag="s")
                nc.tensor.matmul(ps[:, 0:512], wqb, xb[:, js],
                                 start=True, stop=True)
                nc.tensor.matmul(ps[:, 512:1024], wqa, xb[:, js],
                                 start=True, stop=True)
                nc.scalar.copy(phi[:, js], ps[:, 0:512])
                nc.vector.tensor_copy(theta[:, js], ps[:, 512:1024])

            # gT in 65-col slots (col 64 = ones for the row-sum trick);
            # 16 m-tiles batched per PSUM slot for dense PE bursts
            gt = big.tile([P, MT * (CB + 1)], BF16)
            nc.vector.memset(gt, 1.0)
            gt3 = gt.rearrange("p (m c) -> p m c", c=CB + 1)
            for b2 in range(2):
                gp = spool.tile([P, QT], F32, tag="s")
                gp3 = gp.rearrange("p (m c) -> p m c", c=CB)
                for k in range(16):
                    mi = b2 * 16 + k
                    nc.tensor.matmul(
                        gp3[:, k, :], xb[:, mi * 128:(mi + 1) * 128], wg,
                        start=True, stop=True,
                    )
                nc.scalar.copy(
                    gt3[:, b2 * 16:(b2 + 1) * 16, 0:CB], gp3[:, :, :]
                )

            # ---- main attention loop: 512-wide q chunks; per chunk the
            # 32 m-tiles run as 16 pairs, each pair's two score matmuls
            # concurrent in PE row halves; exp split ACT/DVE. The y matmuls
            # are issued YLAG pairs behind the score matmuls: the PE queue
            # is strict FIFO, so a y matmul waiting on its exp would block
            # the (independent) score matmuls issued after it. ----
            YLAG = 3
            for qc in range(8):
                q0 = qc * 512
                yps = ypool.tile([CB + 1, 512], F32, tag="y")
                pbs = {}
                dve_set = DVE_SET if qc % 2 == 0 else DVE_SET2

                def y_mm(pi):
                    pb = pbs.pop(pi)
                    for h in range(2):
                        mi = 2 * pi + h
                        nc.tensor.matmul(
                            yps,
                            gt[:, mi * (CB + 1):(mi + 1) * (CB + 1)],
                            pb[:, h * 512:(h + 1) * 512],
                            start=(pi == 0 and h == 0),
                            stop=(pi == 15 and h == 1),
                        )

                for pi in range(16):
                    sp = spool.tile([P, QT], F32, tag="s")
                    nc.tensor.matmul(
                        sp[:, 0:512],
                        phi[0:CB, (2 * pi) * 128:(2 * pi + 1) * 128],
                        theta[0:CB, q0:q0 + 512], start=True, stop=True,
                    )
                    nc.tensor.matmul(
                        sp[:, 512:1024],
                        phi[CB:P, (2 * pi + 1) * 128:(2 * pi + 2) * 128],
                        theta[CB:P, q0:q0 + 512], start=True, stop=True,
                    )
                    if pi in dve_set:
                        pbi = probs.tile([P, QT], I16, tag="pb")
                        nc.vector.tensor_scalar(
                            pbi, sp, S16, B16,
                            mybir.AluOpType.mult, mybir.AluOpType.add,
                        )
                        pbs[pi] = pbi.bitcast(BF16)
                    else:
                        pb = probs.tile([P, QT], BF16, tag="pb")
                        nc.scalar.activation(
                            pb, sp, mybir.ActivationFunctionType.Exp
                        )
                        pbs[pi] = pb
                    if pi >= YLAG:
                        y_mm(pi - YLAG)
                for pi in range(16 - YLAG, 16):
                    y_mm(pi)

                # ---- epilogue: project UNNORMALIZED y immediately, free all
                # PSUM fast; normalize + residual off the DVE hot path ----
                yu = work.tile([CB + 1, 512], F32R, tag="yu")
                nc.scalar.copy(yu, yps)
                ys = work.tile([1, 512], F32, tag="ys")
                nc.scalar.copy(ys, yps[CB:CB + 1, :])     # frees yps slot
                op = opool.tile([P, 512], F32, tag="op")
                nc.tensor.matmul(op, wlr, yu[0:CB, :], start=True, stop=True)
                rinv = work.tile([1, 512], F32, tag="rinv")
                nc.vector.reciprocal_approx_fast(rinv, ys)
                rb = work.tile([P, 512], F32, tag="rb")
                nc.gpsimd.partition_broadcast(rb, rinv)
                ob = work.tile([P, 512], F32, tag="ob")
                nc.vector.tensor_mul(ob, op, rb)          # frees op slot
                ob2 = work.tile([P, 512], F32, tag="ob2")
                nc.gpsimd.tensor_add(ob2, ob, xb[:, q0:q0 + 512])
                nc.sync.dma_start(out=out_d[:, q0:q0 + 512], in_=ob2)

    nc.finalize()
    return nc


def kernel(x, w_theta, w_phi, w_g, w_last):
    B, C, H, W = x.shape
    N = H * W
    xf = np.ascontiguousarray(x.reshape(B, C, N), dtype=np.float32)
    wqa = np.ascontiguousarray(
        np.concatenate([w_theta.T, w_theta.T], axis=1), dtype=np.float32
    )
    wqb = np.ascontiguousarray(
        np.concatenate([w_phi.T, w_phi.T], axis=1), dtype=np.float32
    )
    wgT = np.ascontiguousarray(w_g.T, dtype=np.float32)
    wl = np.ascontiguousarray(w_last.T, dtype=np.float32)

    if "nc" not in _NC_CACHE:
        _NC_CACHE["nc"] = _build()
    nc = _NC_CACHE["nc"]

    in_maps = [
        {"xb": xf[b], "wqa": wqa, "wqb": wqb, "wgT": wgT, "wl": wl}
        for b in range(B)
    ]
    r = run_bass_kernel_spmd(nc, in_maps, list(range(B)))
    out = np.stack([r.results[b]["out"] for b in range(B)], axis=0)
    return out.reshape(B, C, H, W).astype(np.float32)


# revision 17
# speedup vs baseline: 1.1387x; 1.1387x over previous
"""Trainium2 Bass kernel for the non-local (self-attention over spatial
positions) block.

Per batch b (8 batches -> one per NeuronCore):
    xf    = x[b]                       [C=128, N=4096]
    theta = w_theta @ xf               [64, N]
    phi   = w_phi   @ xf               [64, N]
    g     = w_g     @ xf               [64, N]
    attn  = softmax(theta^T phi)       [N, N]   (softmax over keys m)
    y     = g @ attn^T                 [64, N]
    out   = w_last @ y + xf            [128, N]

Design (per core):
 - scoresT orientation: scoresT[m, q] = sum_k phi[k,m] theta[k,q] with phi
   m-tiles stationary; exp(scoresT) feeds the y matmul directly as the
   moving operand (no transposes).
 - exp is the single-engine bottleneck (N*N = 16.7M elems/core) so it is
   SPLIT across two engines: ~17/32 half-tiles per chunk on ACT (table
   exp, bf16 out) and ~15/32 on DVE via a Schraudolph bit-trick: the bf16
   bits of ~exp(x) are round(128*log2e*x + B16), computed by one
   tensor_scalar (f32 PSUM -> int16 SBUF) and bitcast to bf16. The
   approximation's constant scale factor cancels in softmax.
 - One FLAT software pipeline across all 8 q-chunks: the PE engine queue
   is strict FIFO, so y matmuls are issued YLAG pair-steps behind their
   score matmuls (the exp latency), and chunk epilogues are split in two
   stages interleaved with the next chunk's scores. The PE never waits
   >1us, which also keeps the HAM clock-gate at full rate.
 - No max-subtraction: logits are within +-75; exp fits f32/bf16 range
   and the bit-trick constants are valid to |x|<88.
 - Row sums via a ones column appended to gT (stationary of the y
   matmul); reciprocal via the fast custom-DVE approx on [1,512];
   partition-broadcast and residual-add on GPSIMD.
 - Projections use an f32r copy of x as the moving operand (f32 moving
   operands run at 1/4 rate) and duplicated-column weights so theta/phi
   come out pre-duplicated in both row halves, letting score matmuls for
   two m-tiles run concurrently in disjoint PE row groups.
"""

import sys

import numpy as np

for _p in ("/opt/trn_rl_repo",):
    if _p not in sys.path:
        sys.path.insert(0, _p)

import concourse.bass as bass
from concourse import bacc
import concourse.mybir as mybir
import concourse.tile as tile
from concourse.bass_utils import run_bass_kernel_spmd

F32 = mybir.dt.float32
F32R = mybir.dt.float32r
BF16 = mybir.dt.bfloat16
I16 = mybir.dt.int16

P = 128     # channels C / partition dim
CB = 64     # bottleneck channels
NQ = 4096   # spatial positions (64*64)
MT = 32     # m (key) tiles of 128
YLAG = 3    # y matmuls trail the score matmuls by this many pair-steps
OLAG = 3    # out-projection trails the chunk's last y matmul

LOG2E = 1.4426950408889634
S16 = 128.0 * LOG2E
B16 = 127.0 * 128.0 - 5.60   # Schraudolph bias tuned for min max-rel-err

_NC_CACHE = {}


def _dve_half(qc, hi):
    """Which of a chunk's 32 half-tiles run their exp on the DVE."""
    if hi % 2 == 0:
        return False
    if hi == 31:
        return qc % 2 == 0   # 15 vs 16 DVE halves on alternating chunks
    return True


def _build():
    nc = bacc.Bacc()
    x_in = nc.declare_dram_parameter("xb", [P, NQ], F32, isOutput=False)
    wqa_in = nc.declare_dram_parameter("wqa", [P, P], F32, isOutput=False)
    wqb_in = nc.declare_dram_parameter("wqb", [P, P], F32, isOutput=False)
    wg_in = nc.declare_dram_parameter("wgT", [P, CB], F32, isOutput=False)
    wl_in = nc.declare_dram_parameter("wl", [CB, P], F32, isOutput=False)
    out_d = nc.declare_dram_parameter("out", [P, NQ], F32, isOutput=True)

    with tile.TileContext(nc) as tc:
        with (
            tc.tile_pool(name="const", bufs=1) as const,
            tc.tile_pool(name="big", bufs=1) as big,
            tc.tile_pool(name="work", bufs=2) as work,
            tc.tile_pool(name="probs", bufs=12) as probs,
            tc.tile_pool(name="spool", bufs=5, space="PSUM") as spool,
            tc.tile_pool(name="ypool", bufs=2, space="PSUM") as ypool,
            tc.tile_pool(name="opool", bufs=1, space="PSUM") as opool,
        ):
            # ---- loads ----
            xb = big.tile([P, NQ], F32)
            for j in range(8):
                nc.sync.dma_start(
                    out=xb[:, j * 512:(j + 1) * 512],
                    in_=x_in[:, j * 512:(j + 1) * 512],
                )
            wqa = const.tile([P, P], F32)
            wqb = const.tile([P, P], F32)
            wg = const.tile([P, CB], F32)
            wl = const.tile([CB, P], F32)
            nc.sync.dma_start(out=wqa, in_=wqa_in[:, :])
            nc.sync.dma_start(out=wqb, in_=wqb_in[:, :])
            nc.sync.dma_start(out=wg, in_=wg_in[:, :])
            nc.sync.dma_start(out=wl, in_=wl_in[:, :])
            wlr = const.tile([CB, P], F32R)
            nc.vector.tensor_copy(wlr, wl)
            wqar = const.tile([P, P], F32R)
            wqbr = const.tile([P, P], F32R)
            wgr = const.tile([P, CB], F32R)
            nc.vector.tensor_copy(wqar, wqa)
            nc.vector.tensor_copy(wqbr, wqb)
            nc.vector.tensor_copy(wgr, wg)

            # f32r view of x: moving operand of all projections (f32 moving
            # operands run at 1/4 PE rate; f32r at full rate for >=256 cols)
            xbr = big.tile([P, NQ], F32R)
            for j in range(8):
                js = slice(j * 512, (j + 1) * 512)
                nc.vector.tensor_copy(xbr[:, js], xb[:, js])

            # gT in 65-col slots (col 64 = ones for the row-sum trick);
            # 8 m-tiles batched per PSUM slot; these small matmuls also
            # warm the PE's HAM clock gate before the projections
            gt = big.tile([P, MT * (CB + 1)], BF16)
            nc.vector.memset(gt, 1.0)
            gt3 = gt.rearrange("p (m c) -> p m c", c=CB + 1)
            for b2 in range(4):
                gp = spool.tile([P, 512], F32, tag="s")
                gp3 = gp.rearrange("p (m c) -> p m c", c=CB)
                for k in range(8):
                    mi = b2 * 8 + k
                    nc.tensor.matmul(
                        gp3[:, k, :], xbr[:, mi * 128:(mi + 1) * 128], wgr,
                        start=True, stop=True,
                    )
                nc.scalar.copy(
                    gt3[:, b2 * 8:(b2 + 1) * 8, 0:CB], gp3[:, :, :]
                )

            # ---- projections: wqa/wqb = [w^T | w^T] duplicate theta/phi
            # into both row halves. phi copies on ACT, theta on DVE. ----
            theta = big.tile([P, NQ], F32R)
            phi = big.tile([P, NQ], F32R)
            for j in range(8):
                js = slice(j * 512, (j + 1) * 512)
                pp = spool.tile([P, 512], F32, tag="s")
                nc.tensor.matmul(pp, wqbr, xbr[:, js], start=True, stop=True)
                nc.scalar.copy(phi[:, js], pp)
                pt = spool.tile([P, 512], F32, tag="s")
                nc.tensor.matmul(pt, wqar, xbr[:, js], start=True, stop=True)
                nc.vector.tensor_copy(theta[:, js], pt)

            # ---- flat main pipeline over 8*16 = 128 pair-steps ----
            yps_t = {}      # qc -> y accumulator tile
            pend_y = []     # (qc, pi, [pb0, pb1])
            pend_ep = []    # (due_step, qc)

            def y_mm(qc, pi, pbh):
                for h in range(2):
                    mi = 2 * pi + h
                    nc.tensor.matmul(
                        yps_t[qc],
                        gt[:, mi * (CB + 1):(mi + 1) * (CB + 1)],
                        pbh[h][:, :],
                        start=(pi == 0 and h == 0),
                        stop=(pi == 15 and h == 1),
                    )

            def epilogue_a(qc):
                # runs right after the chunk's last y matmul: drain PSUM
                yu = work.tile([CB + 1, 512], F32R, tag="yu")
                nc.scalar.copy(yu, yps_t[qc])
                ys = work.tile([1, 512], F32, tag="ys")
                nc.scalar.copy(ys, yps_t[qc][CB:CB + 1, :])  # frees yps
                rinv = work.tile([1, 512], F32, tag="rinv")
                nc.vector.reciprocal_approx_fast(rinv, ys)
                rb = work.tile([P, 512], F32, tag="rb")
                nc.gpsimd.partition_broadcast(rb, rinv)
                del yps_t[qc]
                return (yu, rb)

            def epilogue_b(qc, yu, rb):
                # OLAG steps later: yu is drained, project + normalize
                q0 = qc * 512
                op = opool.tile([P, 512], F32, tag="op")
                nc.tensor.matmul(op, wlr, yu[0:CB, :], start=True, stop=True)
                ob = work.tile([P, 512], F32, tag="ob")
                nc.vector.tensor_mul(ob, op, rb)          # frees op slot
                ob2 = work.tile([P, 512], F32, tag="ob2")
                nc.gpsimd.tensor_add(ob2, ob, xb[:, q0:q0 + 512])
                nc.sync.dma_start(out=out_d[:, q0:q0 + 512], in_=ob2)

            def flush(step):
                while pend_ep and pend_ep[0][0] <= step:
                    _, args = pend_ep.pop(0)
                    epilogue_b(*args)
                if pend_y and step - pend_y[0][0] >= YLAG:
                    _, qc, pi, pbh = pend_y.pop(0)
                    y_mm(qc, pi, pbh)
                    if pi == 15:
                        pend_ep.append((step + OLAG, (qc, *epilogue_a(qc))))

            for step in range(128):
                qc, pi = divmod(step, 16)
                q0 = qc * 512
                if pi == 0:
                    yps_t[qc] = ypool.tile(
                        [CB + 1, 512], F32, tag="y", name="yps"
                    )
                pbh = []
                for h in range(2):
                    mi = 2 * pi + h
                    half = (slice(0, CB) if h == 0 else slice(CB, P))
                    sp = spool.tile([P, 512], F32, tag="s")
                    nc.tensor.matmul(
                        sp, phi[half, mi * 128:(mi + 1) * 128],
                        theta[half, q0:q0 + 512], start=True, stop=True,
                    )
                    if _dve_half(qc, mi % 32):
                        pbi = probs.tile([P, 512], I16, tag="pb")
                        nc.vector.tensor_scalar(
                            pbi, sp, S16, B16,
                            mybir.AluOpType.mult, mybir.AluOpType.add,
                        )
                        pbh.append(pbi.bitcast(BF16))
                    else:
                        pb = probs.tile([P, 512], BF16, tag="pb")
                        nc.scalar.activation(
                            pb, sp, mybir.ActivationFunctionType.Exp
                        )
                        pbh.append(pb)
                pend_y.append((step, qc, pi, pbh))
                flush(step)

            # drain
            step = 128
            while pend_y or pend_ep:
                flush(step)
                step += 1

    nc.finalize()
    return nc


def kernel(x, w_theta, w_phi, w_g, w_last):
    B, C, H, W = x.shape
    N = H * W
    xf = np.ascontiguousarray(x.reshape(B, C, N), dtype=np.float32)
    wqa = np.ascontiguousarray(
        np.concatenate([w_theta.T, w_theta.T], axis=1), dtype=np.float32
    )
    wqb = np.ascontiguousarray(
        np.concatenate([w_phi.T, w_phi.T], axis=1), dtype=np.float32
    )
    wgT = np.ascontiguousarray(w_g.T, dtype=np.float32)
    wl = np.ascontiguousarray(w_last.T, dtype=np.float32)

    if "nc" not in _NC_CACHE:
        _NC_CACHE["nc"] = _build()
    nc = _NC_CACHE["nc"]

    in_maps = [
        {"xb": xf[b], "wqa": wqa, "wqb": wqb, "wgT": wgT, "wl": wl}
        for b in range(B)
    ]
    r = run_bass_kernel_spmd(nc, in_maps, list(range(B)))
    out = np.stack([r.results[b]["out"] for b in range(B)], axis=0)
    return out.reshape(B, C, H, W).astype(np.float32)


# revision 20
# speedup vs baseline: 1.7958x; 1.5770x over previous
"""Trainium2 Bass kernel for the non-local (self-attention over spatial
positions) block.

Per batch b (8 batches -> one per NeuronCore):
    xf    = x[b]                       [C=128, N=4096]
    theta = w_theta @ xf               [64, N]
    phi   = w_phi   @ xf               [64, N]
    g     = w_g     @ xf               [64, N]
    attn  = softmax(theta^T phi)       [N, N]   (softmax over keys m)
    y     = g @ attn^T                 [64, N]
    out   = w_last @ y + xf            [128, N]

Design (per core):
 - scoresT orientation: scoresT[m, q] = sum_k phi[k,m] theta[k,q] with phi
   m-tiles stationary; exp(scoresT) feeds the y matmul directly as the
   moving operand (no transposes).
 - exp is the single-engine bottleneck (N*N = 16.7M elems/core) so it is
   SPLIT across two engines: ~17/32 m-tiles per chunk on ACT (table exp,
   bf16 out) and ~15/32 on DVE via a Schraudolph bit-trick: the bf16
   bits of ~exp(x) are round(128*log2e*x + B16), computed by one
   tensor_scalar (f32 PSUM -> int16 SBUF) and bitcast to bf16. The
   approximation's constant scale factor cancels in softmax.
 - q is processed in 4 big-chunks of 1024 (as two 512 halves qA/qB so
   score PSUM tiles stay one bank): one phi LDWEIGHTS pair serves 4
   score streams, and y matmuls go 1024 wide, halving the per-step
   LDWEIGHTS/drain overhead. That keeps PE busy-fraction high enough for
   the HAM clock gate to hold 2.4 GHz.
 - theta/phi are fp16 (10-bit mantissa, noise comparable to f32r) so
   the phi LDWEIGHTS gets fast-weight-load.
 - One FLAT software pipeline across big-chunks: the PE queue is strict
   FIFO, so y matmuls are issued YLAG steps behind their score matmuls
   (covering the exp latency) and epilogues are split in two stages.
 - No max-subtraction: logits are within +-75; exp fits f32/bf16 range
   and the bit-trick constants are valid to |x|<88.
 - Row sums via a ones column appended to gT; reciprocal via the fast
   custom-DVE approx; partition-broadcast and residual-add on GPSIMD.
 - Projections use f32r copies of x / weights as operands (f32 moving
   runs at 1/4 PE rate) with duplicated-column weights so theta/phi come
   out pre-duplicated in both row halves, letting score matmuls for two
   m-tiles run concurrently in disjoint PE row groups.
"""

import sys

import numpy as np

for _p in ("/opt/trn_rl_repo",):
    if _p not in sys.path:
        sys.path.insert(0, _p)

import concourse.bass as bass
from concourse import bacc
import concourse.mybir as mybir
import concourse.tile as tile
from concourse.bass_utils import run_bass_kernel_spmd

F32 = mybir.dt.float32
F32R = mybir.dt.float32r
BF16 = mybir.dt.bfloat16
FP16 = mybir.dt.float16
I16 = mybir.dt.int16

P = 128     # channels C / partition dim
CB = 64     # bottleneck channels
NQ = 4096   # spatial positions (64*64)
MT = 32     # m (key) tiles of 128
YLAG = 3    # y matmuls trail the score matmuls by this many big-steps
OLAG = 3    # out-projection trails the chunk's last y matmul

LOG2E = 1.4426950408889634
S16 = 128.0 * LOG2E
B16 = 127.0 * 128.0 - 5.60   # Schraudolph bias tuned for min max-rel-err

_NC_CACHE = {}


def _dve_mtile(mi):
    """Which m-tiles run their exp on the DVE (15 of 32; ACT gets 17
    plus the per-chunk epilogue copies)."""
    return mi % 2 == 1 and mi != 31


def _build():
    nc = bacc.Bacc()
    x_in = nc.declare_dram_parameter("xb", [P, NQ], F32, isOutput=False)
    wqa_in = nc.declare_dram_parameter("wqa", [P, P], F32, isOutput=False)
    wqb_in = nc.declare_dram_parameter("wqb", [P, P], F32, isOutput=False)
    wg_in = nc.declare_dram_parameter("wgT", [P, CB], F32, isOutput=False)
    wl_in = nc.declare_dram_parameter("wl", [CB, P], F32, isOutput=False)
    out_d = nc.declare_dram_parameter("out", [P, NQ], F32, isOutput=True)

    with tile.TileContext(nc) as tc:
        with (
            tc.tile_pool(name="const", bufs=1) as const,
            tc.tile_pool(name="big", bufs=1) as big,
            tc.tile_pool(name="work", bufs=2) as work,
            tc.tile_pool(name="probs", bufs=10) as probs,
            tc.tile_pool(name="spool", bufs=5, space="PSUM") as spool,
            tc.tile_pool(name="ypool", bufs=1, space="PSUM") as ypool,
            tc.tile_pool(name="opool", bufs=1, space="PSUM") as opool,
        ):
            # ---- loads ----
            xb = big.tile([P, NQ], F32)
            for j in range(8):
                nc.sync.dma_start(
                    out=xb[:, j * 512:(j + 1) * 512],
                    in_=x_in[:, j * 512:(j + 1) * 512],
                )
            wqa = const.tile([P, P], F32)
            wqb = const.tile([P, P], F32)
            wg = const.tile([P, CB], F32)
            wl = const.tile([CB, P], F32)
            nc.sync.dma_start(out=wqa, in_=wqa_in[:, :])
            nc.sync.dma_start(out=wqb, in_=wqb_in[:, :])
            nc.sync.dma_start(out=wg, in_=wg_in[:, :])
            nc.sync.dma_start(out=wl, in_=wl_in[:, :])
            wlr = const.tile([CB, P], F32R)
            nc.vector.tensor_copy(wlr, wl)
            wqar = const.tile([P, P], F32R)
            wqbr = const.tile([P, P], F32R)
            wgr = const.tile([P, CB], F32R)
            nc.vector.tensor_copy(wqar, wqa)
            nc.vector.tensor_copy(wqbr, wqb)
            nc.vector.tensor_copy(wgr, wg)

            # f32r view of x: moving operand of all projections (f32 moving
            # operands run at 1/4 PE rate; f32r at full rate for >=256 cols)
            xbr = big.tile([P, NQ], F32R)
            for j in range(8):
                js = slice(j * 512, (j + 1) * 512)
                nc.vector.tensor_copy(xbr[:, js], xb[:, js])

            # gT in 65-col slots (col 64 = ones for the row-sum trick);
            # 8 m-tiles batched per PSUM slot; these small matmuls also
            # warm the PE's HAM clock gate before the projections
            gt = big.tile([P, MT * (CB + 1)], BF16)
            nc.vector.memset(gt, 1.0)
            gt3 = gt.rearrange("p (m c) -> p m c", c=CB + 1)
            for b2 in range(4):
                gp = spool.tile([P, 512], F32, tag="s")
                gp3 = gp.rearrange("p (m c) -> p m c", c=CB)
                for k in range(8):
                    mi = b2 * 8 + k
                    nc.tensor.matmul(
                        gp3[:, k, :], xbr[:, mi * 128:(mi + 1) * 128], wgr,
                        start=True, stop=True,
                    )
                nc.scalar.copy(
                    gt3[:, b2 * 8:(b2 + 1) * 8, 0:CB], gp3[:, :, :]
                )

            # ---- projections: wqa/wqb = [w^T | w^T] duplicate theta/phi
            # into both row halves. phi copies on ACT, theta on DVE. ----
            theta = big.tile([P, NQ], FP16)
            phi = big.tile([P, NQ], FP16)
            for j in range(8):
                js = slice(j * 512, (j + 1) * 512)
                pp = spool.tile([P, 512], F32, tag="s")
                nc.tensor.matmul(pp, wqbr, xbr[:, js], start=True, stop=True)
                nc.scalar.copy(phi[:, js], pp)
                pt = spool.tile([P, 512], F32, tag="s")
                nc.tensor.matmul(pt, wqar, xbr[:, js], start=True, stop=True)
                nc.vector.tensor_copy(theta[:, js], pt)

            # ---- flat main pipeline: 4 big-chunks x 16 m-pair steps ----
            yps_t = {}      # Qc -> y accumulator tile [65, 1024]
            pend_y = []     # (step, Qc, i, [pb0, pb1])
            pend_ep = []    # (due_step, args)

            def y_mm(Qc, i, pbh):
                for h in range(2):
                    mi = 2 * i + h
                    for v in range(2):
                        nc.tensor.matmul(
                            yps_t[Qc][:, v * 512:(v + 1) * 512],
                            gt[:, mi * (CB + 1):(mi + 1) * (CB + 1)],
                            pbh[h][:, v * 512:(v + 1) * 512],
                            start=(i == 0 and h == 0),
                            stop=(i == 15 and h == 1),
                        )

            def epilogue_a(Qc):
                # right after the chunk's last y matmul: drain PSUM
                yu = work.tile([CB + 1, 1024], F32R, tag="yu")
                nc.scalar.copy(yu, yps_t[Qc])
                ys = work.tile([1, 1024], F32, tag="ys")
                nc.scalar.copy(ys, yps_t[Qc][CB:CB + 1, :])  # frees yps
                rinv = work.tile([1, 1024], F32, tag="rinv")
                nc.vector.reciprocal_approx_fast(rinv, ys)
                rb = work.tile([P, 1024], F32, tag="rb")
                nc.gpsimd.partition_broadcast(rb, rinv)
                del yps_t[Qc]
                return (yu, rb)

            def epilogue_b(Qc, yu, rb):
                # OLAG steps later: project + normalize, one 512 half at
                # a time (the out-projection PSUM tile is one bank)
                q0 = Qc * 1024
                for v in range(2):
                    vs = slice(v * 512, (v + 1) * 512)
                    op = opool.tile([P, 512], F32, tag="op")
                    nc.tensor.matmul(op, wlr, yu[0:CB, vs],
                                     start=True, stop=True)
                    ob = work.tile([P, 512], F32, tag="ob")
                    nc.vector.tensor_mul(ob, op, rb[:, vs])
                    ob2 = work.tile([P, 512], F32, tag="ob2")
                    nc.gpsimd.tensor_add(
                        ob2, ob, xb[:, q0 + v * 512:q0 + (v + 1) * 512]
                    )
                    nc.sync.dma_start(
                        out=out_d[:, q0 + v * 512:q0 + (v + 1) * 512],
                        in_=ob2,
                    )

            def flush(step):
                while pend_ep and pend_ep[0][0] <= step:
                    _, args = pend_ep.pop(0)
                    epilogue_b(*args)
                if pend_y and step - pend_y[0][0] >= YLAG:
                    _, Qc, i, pbh = pend_y.pop(0)
                    y_mm(Qc, i, pbh)
                    if i == 15:
                        pend_ep.append((step + OLAG, (Qc, *epilogue_a(Qc))))

            for step in range(64):
                Qc, i = divmod(step, 16)
                q0 = Qc * 1024
                if i == 0:
                    yps_t[Qc] = ypool.tile(
                        [CB + 1, 1024], F32, tag="y", name="yps"
                    )
                pbh = []
                pbts = []
                for h in range(2):
                    mi = 2 * i + h
                    if _dve_mtile(mi):
                        pbt = probs.tile([P, 1024], I16, tag="pb", name="pbt")
                        pbh.append(pbt.bitcast(BF16))
                    else:
                        pbt = probs.tile([P, 1024], BF16, tag="pb", name="pbt")
                        pbh.append(pbt)
                    pbts.append(pbt)
                # interleave h0/h1 so the two streams overlap in disjoint
                # PE row groups, and qB reuses qA's stationary weights
                sps = [[None, None], [None, None]]
                for v in range(2):   # qA / qB 512-halves
                    vs = slice(q0 + v * 512, q0 + (v + 1) * 512)
                    for h in range(2):
                        mi = 2 * i + h
                        half = (slice(0, CB) if h == 0 else slice(CB, P))
                        sp = spool.tile([P, 512], F32, tag="s")
                        nc.tensor.matmul(
                            sp, phi[half, mi * 128:(mi + 1) * 128],
                            theta[half, vs], start=True, stop=True,
                        )
                        sps[h][v] = sp
                for h in range(2):
                    mi = 2 * i + h
                    for v in range(2):
                        ps = pbts[h][:, v * 512:(v + 1) * 512]
                        if _dve_mtile(mi):
                            nc.vector.tensor_scalar(
                                ps, sps[h][v], S16, B16,
                                mybir.AluOpType.mult, mybir.AluOpType.add,
                            )
                        else:
                            nc.scalar.activation(
                                ps, sps[h][v],
                                mybir.ActivationFunctionType.Exp,
                            )
                pend_y.append((step, Qc, i, pbh))
                flush(step)

            # drain
            step = 64
            while pend_y or pend_ep:
                flush(step)
                step += 1

    nc.finalize()
    return nc


def kernel(x, w_theta, w_phi, w_g, w_last):
    B, C, H, W = x.shape
    N = H * W
    xf = np.ascontiguousarray(x.reshape(B, C, N), dtype=np.float32)
    wqa = np.ascontiguousarray(
        np.concatenate([w_theta.T, w_theta.T], axis=1), dtype=np.float32
    )
    wqb = np.ascontiguousarray(
        np.concatenate([w_phi.T, w_phi.T], axis=1), dtype=np.float32
    )
    wgT = np.ascontiguousarray(w_g.T, dtype=np.float32)
    wl = np.ascontiguousarray(w_last.T, dtype=np.float32)

    if "nc" not in _NC_CACHE:
        _NC_CACHE["nc"] = _build()
    nc = _NC_CACHE["nc"]

    in_maps = [
        {"xb": xf[b], "wqa": wqa, "wqb": wqb, "wgT": wgT, "wl": wl}
        for b in range(B)
    ]
    r = run_bass_kernel_spmd(nc, in_maps, list(range(B)))
    out = np.stack([r.results[b]["out"] for b in range(B)], axis=0)
    return out.reshape(B, C, H, W).astype(np.float32)
